# revision 18
# baseline (speedup 1.0000x reference)
"""Trainium2 Bass kernel for nn_Equivariant_257698037971.

Computes out = relu(x @ lam - (sum_m x) @ gam) for x [B, M, F] = [8192, 512, 64],
lam/gam [F, O] = [64, 128], out [B, M, O] fp32.

Strategy (data-parallel over batch, 8 NeuronCores, no collectives):
  - Host pre-packs x into a transposed bf16 layout xt[p, g*M + m] with
    partition p = r*64 + f holding feature f of batch 2g+r. This removes
    all on-device transposes AND halves input HBM traffic (bf16 vs fp32).
  - Device per batch: out_b^T [o, m] = lam^T @ x_b^T as a single K=128,
    N=512 matmul (lhsT = lam zero-padded block-diagonally so even batches
    pick partitions 0-63 and odd batches 64-127; all operands at
    partition base 0).
  - Pooled term: DVE reduce_sum over the free (m) axis gives s_b[f]; tiny
    K=128 matmuls with zero-padded -gam yield -pooled^T [o] per batch,
    which enters the ReLU as the per-partition bias of the PSUM->SBUF
    activation (out = relu(pm + bias)) - no extra broadcast work.
  - Output stored as bf16 (halves output HBM traffic); host up-casts to
    fp32 and undoes the [o, m] transpose during the final gather.
  - Per chunk of 16 batches: 1x 1MiB load (SP HWDGE), DVE reduces,
    2 pool matmuls + 16 main matmuls, 16 ReLU copies split 12:4 between
    ACT and DVE, 1x 2MiB store (SP HWDGE).
"""

import os
import sys
from contextlib import ExitStack

import numpy as np

sys.path.insert(0, "/opt/trn_rl_repo")

import concourse.bass as bass
import concourse.mybir as mybir
import concourse.tile as tile
from concourse.bass_utils import run_bass_kernel_spmd

B, M, F, O = 8192, 512, 64, 128
N_CORES = 8
SHARD_B = B // N_CORES

G = int(os.environ.get("KERNEL_G", "8"))  # 2-batch groups per chunk
CB = 2 * G                                # batches per chunk

_BF16 = mybir.dt.np(mybir.dt.bfloat16)

# Results of the last run (for test harness introspection).
LAST_RUN = {}


def build_nc(shard_b):
    dt = mybir.dt
    nc = bass.Bass(trn_type="TRN2")

    nchunk = shard_b // CB
    assert shard_b % CB == 0

    xt_d = nc.dram_tensor("xt", [128, (shard_b // 2) * M], dt.bfloat16,
                          kind="ExternalInput")
    lam2_d = nc.dram_tensor("lam2", [128, 2 * O], dt.bfloat16,
                            kind="ExternalInput")
    gam2_d = nc.dram_tensor("gam2n", [128, 2 * O], dt.float32,
                            kind="ExternalInput")
    tinyout = int(os.environ.get("KERNEL_TINYOUT", "0"))
    if tinyout:
        out_d = nc.dram_tensor("out", [nchunk, 128, CB], dt.bfloat16,
                               kind="ExternalOutput")
    else:
        out_d = nc.dram_tensor("out", [nchunk, 128, CB * M], dt.bfloat16,
                               kind="ExternalOutput")

    # xt[p, (c g m)]: chunk c, group g within chunk, position m.
    xt_view = xt_d.rearrange("p (c g m) -> c p g m", c=nchunk, g=G)

    relu_alt = int(os.environ.get("KERNEL_RELU_ALT", "4"))  # 1 of N relus on DVE
    store_eng = os.environ.get("KERNEL_STORE_ENG", "gpsimd")
    reduce_3d = int(os.environ.get("KERNEL_REDUCE_3D", "0"))

    def _bufs(name, dflt):
        return int(os.environ.get(f"KERNEL_BUFS_{name}", str(dflt)))

    with ExitStack() as ctx:
        tc = ctx.enter_context(tile.TileContext(nc))

        cpool = ctx.enter_context(tc.tile_pool(name="consts", bufs=1))
        lam2_s = cpool.tile([128, 2 * O], dt.bfloat16, name="lam2_sb")
        gam2_s = cpool.tile([128, 2 * O], dt.float32, name="gam2_sb")
        nc.sync.dma_start(out=lam2_s[:], in_=lam2_d[:])
        nc.sync.dma_start(out=gam2_s[:], in_=gam2_d[:])

        xpool = ctx.enter_context(tc.tile_pool(name="xin", bufs=_bufs("XIN", 4)))
        spool = ctx.enter_context(tc.tile_pool(name="ssb", bufs=_bufs("S", 3)))
        plpool = ctx.enter_context(tc.tile_pool(name="plsb", bufs=_bufs("PL", 3)))
        opool = ctx.enter_context(tc.tile_pool(name="outsb", bufs=_bufs("OUT", 4)))
        ppsum = ctx.enter_context(
            tc.tile_pool(name="ppsum", bufs=_bufs("PP", 2), space="PSUM"))
        mpsum = ctx.enter_context(
            tc.tile_pool(name="mpsum", bufs=_bufs("MP", 6), space="PSUM"))

        repeat = int(os.environ.get("KERNEL_REPEAT", "1"))
        if int(os.environ.get("KERNEL_NULL", "0")):
            nchunk = 1  # null-work probe: one chunk only (overhead measurement)
        load_eng = {"sync": nc.sync, "scalar": nc.scalar,
                    "gpsimd": nc.gpsimd}[os.environ.get("KERNEL_LOAD_ENG", "sync")]
        for cc in list(range(nchunk)) * repeat:
            x8 = xpool.tile([128, G, M], dt.bfloat16, name="x8")
            load_eng.dma_start(out=x8[:], in_=xt_view[cc])

            # s[p, g] = sum_m x8[p, g, m]  (f-partials of both batches)
            poolsb = plpool.tile([128, CB], dt.float32, name="poolsb")
            if int(os.environ.get("KERNEL_NO_POOL", "0")):
                nc.vector.memset(poolsb[:], 0.0)
            else:
                stile = spool.tile([128, G], dt.float32, name="stile")
                if reduce_3d:
                    nc.vector.reduce_sum(stile[:], x8[:],
                                         axis=mybir.AxisListType.X)
                else:
                    for g in range(G):
                        nc.vector.reduce_sum(stile[:, g:g + 1], x8[:, g, :],
                                             axis=mybir.AxisListType.X)

                # pp_r[o, g] = -pooled_{2g+r}[o]; K=128 with zero-padded -gam
                # so each half of the partitions contributes to exactly one r.
                for r in (0, 1):
                    pp = ppsum.tile([128, G], dt.float32, name="pp")
                    nc.tensor.matmul(pp[:], lhsT=gam2_s[:, O * r:O * (r + 1)],
                                     rhs=stile[:], start=True, stop=True)
                    nc.vector.tensor_copy(poolsb[:, G * r:G * (r + 1)], pp[:])

            outc = opool.tile([128, CB * M], dt.bfloat16, name="outc")
            for g in range(G):
                for r in (0, 1):
                    j = 2 * g + r
                    pm = mpsum.tile([128, M], dt.float32, name="pm")
                    nc.tensor.matmul(
                        pm[:], lhsT=lam2_s[:, O * r:O * (r + 1)],
                        rhs=x8[:, g, :],
                        start=True, stop=True,
                    )
                    bias = poolsb[:, G * r + g:G * r + g + 1]
                    if j % relu_alt == relu_alt - 1:
                        nc.vector.tensor_scalar(
                            outc[:, M * j:M * (j + 1)], pm[:], bias, 0.0,
                            mybir.AluOpType.add, mybir.AluOpType.max,
                        )
                    else:
                        nc.scalar.activation(
                            outc[:, M * j:M * (j + 1)], pm[:],
                            mybir.ActivationFunctionType.Relu, bias=bias,
                        )
            if store_eng == "alt":
                eng = nc.sync if cc % 2 == 0 else nc.scalar
            elif store_eng == "galt":
                eng = nc.gpsimd if cc % 2 == 0 else nc.scalar
            elif store_eng == "gsalt":
                eng = nc.gpsimd if cc % 2 == 0 else nc.sync
            elif store_eng == "gpsimd":
                eng = nc.gpsimd
            else:
                eng = nc.sync
            if tinyout:
                pass  # probe mode: no store at all
            else:
                eng.dma_start(out=out_d[cc], in_=outc[:])

    _split_multi_waits(nc)
    return nc


def _split_multi_waits(nc):
    """Walrus can only encode ONE sync wait per TPB instruction (the ISA
    EVENTS struct has a single wait slot); Tile sometimes attaches 2+.
    Hoist all-but-one wait into standalone EventSemaphore instructions
    placed immediately before, on the same (in-order) engine queue."""
    n_split = 0
    for fn in nc.m.functions:
        for blk in fn.blocks:
            out = []
            changed = False
            for inst in blk.instructions:
                si = inst.sync_info
                if (
                    si is not None
                    and si.on_wait
                    and len(si.on_wait) > 1
                    and not isinstance(inst, mybir.InstEventSemaphore)
                ):
                    for w in si.on_wait[:-1]:
                        ev = mybir.InstEventSemaphore(
                            name=nc.get_next_instruction_name(),
                            opcode="EventSemaphore",
                            engine=inst.engine,
                            sync_info=mybir.SyncInfo(on_wait=[w], on_update=[]),
                            bass_nofuse=True,
                        )
                        nc.inst_map[ev.name] = ev
                        out.append(ev)
                        n_split += 1
                    inst.sync_info = mybir.SyncInfo(
                        on_wait=[si.on_wait[-1]], on_update=list(si.on_update)
                    )
                    changed = True
                out.append(inst)
            if changed:
                blk.instructions = out
    return n_split


def _cpu_jax():
    import jax
    return jax, jax.devices("cpu")[0]


def prep_inputs(x, lam, gam, shard_b=SHARD_B):
    """Host-side packing. Returns per-core input arrays with a leading
    [N_CORES] axis, keyed by DRAM tensor name."""
    n_cores = x.shape[0] // shard_b
    jax, cpu = _cpu_jax()
    import jax.numpy as jnp

    with jax.default_device(cpu):
        xj = jnp.asarray(x, dtype=jnp.bfloat16)
        # [cores, shard_b/2 groups, 2, M, F] -> [cores, 2, F, groups, M]
        xt = xj.reshape(n_cores, shard_b // 2, 2, M, F)
        xt = jnp.transpose(xt, (0, 2, 4, 1, 3))
        xt = xt.reshape(n_cores, 128, (shard_b // 2) * M)
        xt = np.asarray(jax.block_until_ready(xt))

    # lam2[p, O*r + o] = lam[p - 64r, o] for p//64 == r, else 0.
    lam2 = np.zeros((128, 2 * O), np.float32)
    lam2[0:64, 0:O] = lam
    lam2[64:128, O:2 * O] = lam
    lam2 = lam2.astype(_BF16)
    gam2n = np.zeros((128, 2 * O), np.float32)
    gam2n[0:64, 0:O] = -gam
    gam2n[64:128, O:2 * O] = -gam
    return {
        "xt": xt,
        "lam2": np.broadcast_to(lam2, (n_cores,) + lam2.shape),
        "gam2n": np.broadcast_to(gam2n, (n_cores,) + gam2n.shape),
    }


def gather_outputs(outd):
    """outd: [n_cores, nchunk, 128, CB*M] bf16 (transposed layout) ->
    full [B', M, O] fp32 output."""
    n_cores = outd.shape[0]
    nchunk = outd.shape[1]
    jax, cpu = _cpu_jax()
    import jax.numpy as jnp

    with jax.default_device(cpu):
        oj = jnp.asarray(outd)
        oj = oj.reshape(n_cores, nchunk, O, CB, M)
        oj = jnp.transpose(oj, (0, 1, 3, 4, 2))  # -> [cores, chunk, j, m, o]
        oj = oj.reshape(n_cores * nchunk * CB, M, O).astype(jnp.float32)
        return np.asarray(jax.block_until_ready(oj))


def kernel(x, lam, gam):
    x = np.asarray(x, dtype=np.float32)
    lam = np.asarray(lam, dtype=np.float32)
    gam = np.asarray(gam, dtype=np.float32)
    shard_b = x.shape[0] // N_CORES
    assert x.shape[0] % N_CORES == 0

    nc = build_nc(shard_b)
    per_core = prep_inputs(x, lam, gam, shard_b)
    in_maps = [
        {name: arr[c] for name, arr in per_core.items()} for c in range(N_CORES)
    ]
    trace = bool(int(os.environ.get("KERNEL_TRACE", "0")))
    res = run_bass_kernel_spmd(
        nc, in_maps, core_ids=list(range(N_CORES)), trace=trace
    )
    LAST_RUN["exec_time_ns"] = res.exec_time_ns
    LAST_RUN["mean_exec_time_ns"] = res.mean_exec_time_ns
    outd = np.stack([r["out"] for r in res.results], axis=0)
    return gather_outputs(outd)


# revision 19
# speedup vs baseline: 1.2324x; 1.2324x over previous
"""Trainium2 Bass kernel for nn_Equivariant_257698037971.

Computes out = relu(x @ lam - (sum_m x) @ gam) for x [B, M, F] = [8192, 512, 64],
lam/gam [F, O] = [64, 128], out [B, M, O] fp32.

Strategy (data-parallel over batch, 8 NeuronCores, no collectives):
  - Host pre-packs x into a transposed bf16 layout xt[p, g*M + m] with
    partition p = r*64 + f holding feature f of batch 2g+r. This removes
    all on-device transposes AND halves input HBM traffic (bf16 vs fp32).
  - Device per batch: out_b^T [o, m] = lam^T @ x_b^T as a single K=128,
    N=512 matmul (lhsT = lam zero-padded block-diagonally so even batches
    pick partitions 0-63 and odd batches 64-127; all operands at
    partition base 0).
  - Pooled term: DVE reduce_sum over the free (m) axis gives s_b[f]; tiny
    K=128 matmuls with zero-padded -gam yield -pooled^T [o] per batch,
    which enters the ReLU as the per-partition bias of the PSUM->SBUF
    activation (out = relu(pm + bias)) - no extra broadcast work.
  - Output stored as bf16 (halves output HBM traffic); host up-casts to
    fp32 and undoes the [o, m] transpose during the final gather.
  - Per chunk of 16 batches: 1x 1MiB load (SP HWDGE ring), DVE reduces,
    2 pool matmuls + 16 main matmuls, 16 ReLU copies split 12:4 between
    ACT and DVE, 1x 2MiB store (SWDGE ring, so load/store issue paths
    don't serialize on one queue).
"""

import os
import sys
from contextlib import ExitStack

import numpy as np

sys.path.insert(0, "/opt/trn_rl_repo")

import concourse.bass as bass
import concourse.mybir as mybir
import concourse.tile as tile
from concourse.bass_utils import run_bass_kernel_spmd

B, M, F, O = 8192, 512, 64, 128
N_CORES = 8
SHARD_B = B // N_CORES

G = int(os.environ.get("KERNEL_G", "8"))  # 2-batch groups per chunk
CB = 2 * G                                # batches per chunk

_BF16 = mybir.dt.np(mybir.dt.bfloat16)

# Results of the last run (for test harness introspection).
LAST_RUN = {}


def build_nc(shard_b):
    dt = mybir.dt
    nc = bass.Bass(trn_type="TRN2")

    nchunk = shard_b // CB
    assert shard_b % CB == 0

    xt_d = nc.dram_tensor("xt", [128, (shard_b // 2) * M], dt.bfloat16,
                          kind="ExternalInput")
    lam2_d = nc.dram_tensor("lam2", [128, 2 * O], dt.bfloat16,
                            kind="ExternalInput")
    gam2_d = nc.dram_tensor("gam2n", [128, 2 * O], dt.float32,
                            kind="ExternalInput")
    tinyout = int(os.environ.get("KERNEL_TINYOUT", "0"))
    if tinyout:
        out_d = nc.dram_tensor("out", [nchunk, 128, CB], dt.bfloat16,
                               kind="ExternalOutput")
    else:
        out_d = nc.dram_tensor("out", [nchunk, 128, CB * M], dt.bfloat16,
                               kind="ExternalOutput")

    # xt[p, (c g m)]: chunk c, group g within chunk, position m.
    xt_view = xt_d.rearrange("p (c g m) -> c p g m", c=nchunk, g=G)

    relu_alt = int(os.environ.get("KERNEL_RELU_ALT", "4"))  # 1 of N relus on DVE
    store_eng = os.environ.get("KERNEL_STORE_ENG", "gpsimd")
    reduce_3d = int(os.environ.get("KERNEL_REDUCE_3D", "0"))

    def _bufs(name, dflt):
        return int(os.environ.get(f"KERNEL_BUFS_{name}", str(dflt)))

    with ExitStack() as ctx:
        tc = ctx.enter_context(tile.TileContext(nc))

        cpool = ctx.enter_context(tc.tile_pool(name="consts", bufs=1))
        lam2_s = cpool.tile([128, 2 * O], dt.bfloat16, name="lam2_sb")
        gam2_s = cpool.tile([128, 2 * O], dt.float32, name="gam2_sb")
        nc.sync.dma_start(out=lam2_s[:], in_=lam2_d[:])
        nc.sync.dma_start(out=gam2_s[:], in_=gam2_d[:])

        xpool = ctx.enter_context(tc.tile_pool(name="xin", bufs=_bufs("XIN", 4)))
        spool = ctx.enter_context(tc.tile_pool(name="ssb", bufs=_bufs("S", 3)))
        plpool = ctx.enter_context(tc.tile_pool(name="plsb", bufs=_bufs("PL", 3)))
        opool = ctx.enter_context(tc.tile_pool(name="outsb", bufs=_bufs("OUT", 4)))
        ppsum = ctx.enter_context(
            tc.tile_pool(name="ppsum", bufs=_bufs("PP", 2), space="PSUM"))
        mpsum = ctx.enter_context(
            tc.tile_pool(name="mpsum", bufs=_bufs("MP", 6), space="PSUM"))

        repeat = int(os.environ.get("KERNEL_REPEAT", "1"))
        if int(os.environ.get("KERNEL_NULL", "0")):
            nchunk = 1  # null-work probe: one chunk only (overhead measurement)
        load_eng = {"sync": nc.sync, "scalar": nc.scalar,
                    "gpsimd": nc.gpsimd}[os.environ.get("KERNEL_LOAD_ENG", "sync")]
        for cc in list(range(nchunk)) * repeat:
            x8 = xpool.tile([128, G, M], dt.bfloat16, name="x8")
            load_eng.dma_start(out=x8[:], in_=xt_view[cc])

            # s[p, g] = sum_m x8[p, g, m]  (f-partials of both batches)
            poolsb = plpool.tile([128, CB], dt.float32, name="poolsb")
            if int(os.environ.get("KERNEL_NO_POOL", "0")):
                nc.vector.memset(poolsb[:], 0.0)
            else:
                stile = spool.tile([128, G], dt.float32, name="stile")
                if reduce_3d:
                    nc.vector.reduce_sum(stile[:], x8[:],
                                         axis=mybir.AxisListType.X)
                else:
                    for g in range(G):
                        nc.vector.reduce_sum(stile[:, g:g + 1], x8[:, g, :],
                                             axis=mybir.AxisListType.X)

                # pp_r[o, g] = -pooled_{2g+r}[o]; K=128 with zero-padded -gam
                # so each half of the partitions contributes to exactly one r.
                for r in (0, 1):
                    pp = ppsum.tile([128, G], dt.float32, name="pp")
                    nc.tensor.matmul(pp[:], lhsT=gam2_s[:, O * r:O * (r + 1)],
                                     rhs=stile[:], start=True, stop=True)
                    nc.vector.tensor_copy(poolsb[:, G * r:G * (r + 1)], pp[:])

            outc = opool.tile([128, CB * M], dt.bfloat16, name="outc")
            for g in range(G):
                for r in (0, 1):
                    j = 2 * g + r
                    pm = mpsum.tile([128, M], dt.float32, name="pm")
                    nc.tensor.matmul(
                        pm[:], lhsT=lam2_s[:, O * r:O * (r + 1)],
                        rhs=x8[:, g, :],
                        start=True, stop=True,
                    )
                    bias = poolsb[:, G * r + g:G * r + g + 1]
                    if j % relu_alt == relu_alt - 1:
                        nc.vector.tensor_scalar(
                            outc[:, M * j:M * (j + 1)], pm[:], bias, 0.0,
                            mybir.AluOpType.add, mybir.AluOpType.max,
                        )
                    else:
                        nc.scalar.activation(
                            outc[:, M * j:M * (j + 1)], pm[:],
                            mybir.ActivationFunctionType.Relu, bias=bias,
                        )
            if store_eng == "alt":
                eng = nc.sync if cc % 2 == 0 else nc.scalar
            elif store_eng == "galt":
                eng = nc.gpsimd if cc % 2 == 0 else nc.scalar
            elif store_eng == "gsalt":
                eng = nc.gpsimd if cc % 2 == 0 else nc.sync
            elif store_eng == "gpsimd":
                eng = nc.gpsimd
            else:
                eng = nc.sync
            if tinyout:
                pass  # probe mode: no store at all
            else:
                eng.dma_start(out=out_d[cc], in_=outc[:])

    _split_multi_waits(nc)
    return nc


def _split_multi_waits(nc):
    """Walrus can only encode ONE sync wait per TPB instruction (the ISA
    EVENTS struct has a single wait slot); Tile sometimes attaches 2+.
    Hoist all-but-one wait into standalone EventSemaphore instructions
    placed immediately before, on the same (in-order) engine queue."""
    n_split = 0
    for fn in nc.m.functions:
        for blk in fn.blocks:
            out = []
            changed = False
            for inst in blk.instructions:
                si = inst.sync_info
                if (
                    si is not None
                    and si.on_wait
                    and len(si.on_wait) > 1
                    and not isinstance(inst, mybir.InstEventSemaphore)
                ):
                    for w in si.on_wait[:-1]:
                        ev = mybir.InstEventSemaphore(
                            name=nc.get_next_instruction_name(),
                            opcode="EventSemaphore",
                            engine=inst.engine,
                            sync_info=mybir.SyncInfo(on_wait=[w], on_update=[]),
                            bass_nofuse=True,
                        )
                        nc.inst_map[ev.name] = ev
                        out.append(ev)
                        n_split += 1
                    inst.sync_info = mybir.SyncInfo(
                        on_wait=[si.on_wait[-1]], on_update=list(si.on_update)
                    )
                    changed = True
                out.append(inst)
            if changed:
                blk.instructions = out
    return n_split


def _cpu_jax():
    import jax
    return jax, jax.devices("cpu")[0]


def prep_inputs(x, lam, gam, shard_b=SHARD_B):
    """Host-side packing. Returns per-core input arrays with a leading
    [N_CORES] axis, keyed by DRAM tensor name."""
    n_cores = x.shape[0] // shard_b
    jax, cpu = _cpu_jax()
    import jax.numpy as jnp

    with jax.default_device(cpu):
        xj = jnp.asarray(x, dtype=jnp.bfloat16)
        # [cores, shard_b/2 groups, 2, M, F] -> [cores, 2, F, groups, M]
        xt = xj.reshape(n_cores, shard_b // 2, 2, M, F)
        xt = jnp.transpose(xt, (0, 2, 4, 1, 3))
        xt = xt.reshape(n_cores, 128, (shard_b // 2) * M)
        xt = np.asarray(jax.block_until_ready(xt))

    # lam2[p, O*r + o] = lam[p - 64r, o] for p//64 == r, else 0.
    lam2 = np.zeros((128, 2 * O), np.float32)
    lam2[0:64, 0:O] = lam
    lam2[64:128, O:2 * O] = lam
    lam2 = lam2.astype(_BF16)
    gam2n = np.zeros((128, 2 * O), np.float32)
    gam2n[0:64, 0:O] = -gam
    gam2n[64:128, O:2 * O] = -gam
    return {
        "xt": xt,
        "lam2": np.broadcast_to(lam2, (n_cores,) + lam2.shape),
        "gam2n": np.broadcast_to(gam2n, (n_cores,) + gam2n.shape),
    }


def gather_outputs(outd):
    """outd: [n_cores, nchunk, 128, CB*M] bf16 (transposed layout) ->
    full [B', M, O] fp32 output."""
    n_cores = outd.shape[0]
    nchunk = outd.shape[1]
    jax, cpu = _cpu_jax()
    import jax.numpy as jnp

    with jax.default_device(cpu):
        oj = jnp.asarray(outd)
        oj = oj.reshape(n_cores, nchunk, O, CB, M)
        oj = jnp.transpose(oj, (0, 1, 3, 4, 2))  # -> [cores, chunk, j, m, o]
        oj = oj.reshape(n_cores * nchunk * CB, M, O).astype(jnp.float32)
        return np.asarray(jax.block_until_ready(oj))


def kernel(x, lam, gam):
    x = np.asarray(x, dtype=np.float32)
    lam = np.asarray(lam, dtype=np.float32)
    gam = np.asarray(gam, dtype=np.float32)
    shard_b = x.shape[0] // N_CORES
    assert x.shape[0] % N_CORES == 0

    nc = build_nc(shard_b)
    per_core = prep_inputs(x, lam, gam, shard_b)
    in_maps = [
        {name: arr[c] for name, arr in per_core.items()} for c in range(N_CORES)
    ]
    trace = bool(int(os.environ.get("KERNEL_TRACE", "0")))
    res = run_bass_kernel_spmd(
        nc, in_maps, core_ids=list(range(N_CORES)), trace=trace
    )
    LAST_RUN["exec_time_ns"] = res.exec_time_ns
    LAST_RUN["mean_exec_time_ns"] = res.mean_exec_time_ns
    outd = np.stack([r["out"] for r in res.results], axis=0)
    return gather_outputs(outd)


# revision 20
# speedup vs baseline: 1.2694x; 1.0300x over previous
"""Trainium2 Bass kernel for nn_Equivariant_257698037971.

Computes out = relu(x @ lam - (sum_m x) @ gam) for x [B, M, F] = [8192, 512, 64],
lam/gam [F, O] = [64, 128], out [B, M, O] fp32.

Strategy (data-parallel over batch, 8 NeuronCores, no collectives):
  - Host pre-packs x into a transposed bf16 layout xt[p, g*M + m] with
    partition p = r*64 + f holding feature f of batch 2g+r. This removes
    all on-device transposes AND halves input HBM traffic (bf16 vs fp32).
  - Device per batch: out_b^T [o, m] = lam^T @ x_b^T as a single K=128,
    N=512 matmul (lhsT = lam zero-padded block-diagonally so even batches
    pick partitions 0-63 and odd batches 64-127; all operands at
    partition base 0).
  - Pooled term: DVE reduce_sum over the free (m) axis gives s_b[f]; tiny
    K=128 matmuls with zero-padded -gam yield -pooled^T [o] per batch,
    which enters the ReLU as the per-partition bias of the PSUM->SBUF
    activation (out = relu(pm + bias)) - no extra broadcast work.
  - Output stored as bf16 (halves output HBM traffic); host up-casts to
    fp32 and undoes the [o, m] transpose during the final gather.
  - Per chunk of 16 batches: 1x 1MiB load (SP HWDGE ring), DVE reduces,
    2 pool matmuls + 16 main matmuls, 16 ReLU copies split 12:4 between
    ACT and DVE, 1x 2MiB store (SWDGE ring, so load/store issue paths
    don't serialize on one queue).
"""

import os
import sys
from contextlib import ExitStack

import numpy as np

sys.path.insert(0, "/opt/trn_rl_repo")

import concourse.bass as bass
import concourse.mybir as mybir
import concourse.tile as tile
from concourse.bass_utils import run_bass_kernel_spmd

B, M, F, O = 8192, 512, 64, 128
N_CORES = 8
SHARD_B = B // N_CORES

G = int(os.environ.get("KERNEL_G", "8"))  # 2-batch groups per chunk
CB = 2 * G                                # batches per chunk

_BF16 = mybir.dt.np(mybir.dt.bfloat16)

# Results of the last run (for test harness introspection).
LAST_RUN = {}


def build_nc(shard_b):
    dt = mybir.dt
    nc = bass.Bass(trn_type="TRN2")

    nchunk = shard_b // CB
    assert shard_b % CB == 0

    xt_d = nc.dram_tensor("xt", [128, (shard_b // 2) * M], dt.bfloat16,
                          kind="ExternalInput")
    lam2_d = nc.dram_tensor("lam2", [128, 2 * O], dt.bfloat16,
                            kind="ExternalInput")
    gam2_d = nc.dram_tensor("gam2n", [128, 2 * O], dt.float32,
                            kind="ExternalInput")
    tinyout = int(os.environ.get("KERNEL_TINYOUT", "0"))
    if tinyout:
        out_d = nc.dram_tensor("out", [nchunk, 128, CB], dt.bfloat16,
                               kind="ExternalOutput")
    else:
        out_d = nc.dram_tensor("out", [nchunk, 128, CB * M], dt.bfloat16,
                               kind="ExternalOutput")

    # xt[p, (c g m)]: chunk c, group g within chunk, position m.
    xt_view = xt_d.rearrange("p (c g m) -> c p g m", c=nchunk, g=G)

    relu_alt = int(os.environ.get("KERNEL_RELU_ALT", "4"))  # 1 of N relus on DVE
    store_eng = os.environ.get("KERNEL_STORE_ENG", "gpsimd")
    reduce_3d = int(os.environ.get("KERNEL_REDUCE_3D", "0"))

    def _bufs(name, dflt):
        return int(os.environ.get(f"KERNEL_BUFS_{name}", str(dflt)))

    with ExitStack() as ctx:
        tc = ctx.enter_context(tile.TileContext(nc))

        cpool = ctx.enter_context(tc.tile_pool(name="consts", bufs=1))
        lam2_s = cpool.tile([128, 2 * O], dt.bfloat16, name="lam2_sb")
        gam2_s = cpool.tile([128, 2 * O], dt.float32, name="gam2_sb")
        nc.sync.dma_start(out=lam2_s[:], in_=lam2_d[:])
        nc.sync.dma_start(out=gam2_s[:], in_=gam2_d[:])

        xpool = ctx.enter_context(tc.tile_pool(name="xin", bufs=_bufs("XIN", 4)))
        spool = ctx.enter_context(tc.tile_pool(name="ssb", bufs=_bufs("S", 3)))
        plpool = ctx.enter_context(tc.tile_pool(name="plsb", bufs=_bufs("PL", 3)))
        opool = ctx.enter_context(tc.tile_pool(name="outsb", bufs=_bufs("OUT", 4)))
        ppsum = ctx.enter_context(
            tc.tile_pool(name="ppsum", bufs=_bufs("PP", 2), space="PSUM"))
        mpsum = ctx.enter_context(
            tc.tile_pool(name="mpsum", bufs=_bufs("MP", 6), space="PSUM"))

        repeat = int(os.environ.get("KERNEL_REPEAT", "1"))
        if int(os.environ.get("KERNEL_NULL", "0")):
            nchunk = 1  # null-work probe: one chunk only (overhead measurement)
        load_eng = {"sync": nc.sync, "scalar": nc.scalar,
                    "gpsimd": nc.gpsimd}[os.environ.get("KERNEL_LOAD_ENG", "sync")]
        for cc in list(range(nchunk)) * repeat:
            x8 = xpool.tile([128, G, M], dt.bfloat16, name="x8")
            load_eng.dma_start(out=x8[:], in_=xt_view[cc])

            # s[p, g] = sum_m x8[p, g, m]  (f-partials of both batches)
            poolsb = plpool.tile([128, CB], dt.float32, name="poolsb")
            if int(os.environ.get("KERNEL_NO_POOL", "0")):
                nc.vector.memset(poolsb[:], 0.0)
            else:
                stile = spool.tile([128, G], dt.float32, name="stile")
                if reduce_3d:
                    nc.vector.reduce_sum(stile[:], x8[:],
                                         axis=mybir.AxisListType.X)
                else:
                    for g in range(G):
                        nc.vector.reduce_sum(stile[:, g:g + 1], x8[:, g, :],
                                             axis=mybir.AxisListType.X)

                # pp_r[o, g] = -pooled_{2g+r}[o]; K=128 with zero-padded -gam
                # so each half of the partitions contributes to exactly one r.
                for r in (0, 1):
                    pp = ppsum.tile([128, G], dt.float32, name="pp")
                    nc.tensor.matmul(pp[:], lhsT=gam2_s[:, O * r:O * (r + 1)],
                                     rhs=stile[:], start=True, stop=True)
                    nc.vector.tensor_copy(poolsb[:, G * r:G * (r + 1)], pp[:])

            outc = opool.tile([128, CB * M], dt.bfloat16, name="outc")
            for g in range(G):
                for r in (0, 1):
                    j = 2 * g + r
                    pm = mpsum.tile([128, M], dt.float32, name="pm")
                    nc.tensor.matmul(
                        pm[:], lhsT=lam2_s[:, O * r:O * (r + 1)],
                        rhs=x8[:, g, :],
                        start=True, stop=True,
                    )
                    bias = poolsb[:, G * r + g:G * r + g + 1]
                    if j % relu_alt == relu_alt - 1:
                        nc.vector.tensor_scalar(
                            outc[:, M * j:M * (j + 1)], pm[:], bias, 0.0,
                            mybir.AluOpType.add, mybir.AluOpType.max,
                        )
                    else:
                        nc.scalar.activation(
                            outc[:, M * j:M * (j + 1)], pm[:],
                            mybir.ActivationFunctionType.Relu, bias=bias,
                        )
            if store_eng == "alt":
                eng = nc.sync if cc % 2 == 0 else nc.scalar
            elif store_eng == "galt":
                eng = nc.gpsimd if cc % 2 == 0 else nc.scalar
            elif store_eng == "gsalt":
                eng = nc.gpsimd if cc % 2 == 0 else nc.sync
            elif store_eng == "gpsimd":
                eng = nc.gpsimd
            else:
                eng = nc.sync
            if tinyout:
                pass  # probe mode: no store at all
            else:
                eng.dma_start(out=out_d[cc], in_=outc[:])

    _split_multi_waits(nc)
    return nc


def _split_multi_waits(nc):
    """Walrus can only encode ONE sync wait per TPB instruction (the ISA
    EVENTS struct has a single wait slot); Tile sometimes attaches 2+.
    Hoist all-but-one wait into standalone EventSemaphore instructions
    placed immediately before, on the same (in-order) engine queue."""
    n_split = 0
    for fn in nc.m.functions:
        for blk in fn.blocks:
            out = []
            changed = False
            for inst in blk.instructions:
                si = inst.sync_info
                if (
                    si is not None
                    and si.on_wait
                    and len(si.on_wait) > 1
                    and not isinstance(inst, mybir.InstEventSemaphore)
                ):
                    for w in si.on_wait[:-1]:
                        ev = mybir.InstEventSemaphore(
                            name=nc.get_next_instruction_name(),
                            opcode="EventSemaphore",
                            engine=inst.engine,
                            sync_info=mybir.SyncInfo(on_wait=[w], on_update=[]),
                            bass_nofuse=True,
                        )
                        nc.inst_map[ev.name] = ev
                        out.append(ev)
                        n_split += 1
                    inst.sync_info = mybir.SyncInfo(
                        on_wait=[si.on_wait[-1]], on_update=list(si.on_update)
                    )
                    changed = True
                out.append(inst)
            if changed:
                blk.instructions = out
    return n_split


def _cpu_jax():
    import jax
    return jax, jax.devices("cpu")[0]


def prep_inputs(x, lam, gam, shard_b=SHARD_B):
    """Host-side packing. Returns per-core input arrays with a leading
    [N_CORES] axis, keyed by DRAM tensor name."""
    n_cores = x.shape[0] // shard_b
    jax, cpu = _cpu_jax()
    import jax.numpy as jnp

    with jax.default_device(cpu):
        xj = jnp.asarray(x, dtype=jnp.bfloat16)
        # [cores, shard_b/2 groups, 2, M, F] -> [cores, 2, F, groups, M]
        xt = xj.reshape(n_cores, shard_b // 2, 2, M, F)
        xt = jnp.transpose(xt, (0, 2, 4, 1, 3))
        xt = xt.reshape(n_cores, 128, (shard_b // 2) * M)
        xt = np.asarray(jax.block_until_ready(xt))

    # lam2[p, O*r + o] = lam[p - 64r, o] for p//64 == r, else 0.
    lam2 = np.zeros((128, 2 * O), np.float32)
    lam2[0:64, 0:O] = lam
    lam2[64:128, O:2 * O] = lam
    lam2 = lam2.astype(_BF16)
    gam2n = np.zeros((128, 2 * O), np.float32)
    gam2n[0:64, 0:O] = -gam
    gam2n[64:128, O:2 * O] = -gam
    return {
        "xt": xt,
        "lam2": np.broadcast_to(lam2, (n_cores,) + lam2.shape),
        "gam2n": np.broadcast_to(gam2n, (n_cores,) + gam2n.shape),
    }


def gather_outputs(outd):
    """outd: [n_cores, nchunk, 128, CB*M] bf16 (transposed layout) ->
    full [B', M, O] fp32 output."""
    n_cores = outd.shape[0]
    nchunk = outd.shape[1]
    jax, cpu = _cpu_jax()
    import jax.numpy as jnp

    with jax.default_device(cpu):
        oj = jnp.asarray(outd)
        oj = oj.reshape(n_cores, nchunk, O, CB, M)
        oj = jnp.transpose(oj, (0, 1, 3, 4, 2))  # -> [cores, chunk, j, m, o]
        oj = oj.reshape(n_cores * nchunk * CB, M, O).astype(jnp.float32)
        return np.asarray(jax.block_until_ready(oj))


def kernel(x, lam, gam):
    x = np.asarray(x, dtype=np.float32)
    lam = np.asarray(lam, dtype=np.float32)
    gam = np.asarray(gam, dtype=np.float32)
    shard_b = x.shape[0] // N_CORES
    assert x.shape[0] % N_CORES == 0

    nc = build_nc(shard_b)
    per_core = prep_inputs(x, lam, gam, shard_b)
    in_maps = [
        {name: arr[c] for name, arr in per_core.items()} for c in range(N_CORES)
    ]
    trace = bool(int(os.environ.get("KERNEL_TRACE", "0")))
    # The axon-proxied first execution occasionally dies with a transient
    # NRT_EXEC_UNIT_UNRECOVERABLE / mesh-desync; the device recovers on a
    # fresh attempt, so retry a couple of times before giving up.
    last_exc = None
    for attempt in range(3):
        try:
            res = run_bass_kernel_spmd(
                nc, in_maps, core_ids=list(range(N_CORES)), trace=trace
            )
            break
        except Exception as e:  # noqa: BLE001 - retrying runtime flakes
            last_exc = e
            LAST_RUN["retries"] = attempt + 1
            if attempt == 2:
                raise
            import time as _time
            _time.sleep(10.0)
    LAST_RUN["exec_time_ns"] = res.exec_time_ns
    LAST_RUN["mean_exec_time_ns"] = res.mean_exec_time_ns
    outd = np.stack([r["out"] for r in res.results], axis=0)
    return gather_outputs(outd)
